# revision 1
# baseline (speedup 1.0000x reference)
"""Causal self-attention (B=2, T=2048, C=1024, H=16) on 8 trn2 NeuronCores.

Sharding: core c = (b, g) with b = c // 4 (batch), g = c % 4 (head-group of 4
heads = 256 dims). Per core:
  1. QKV projection from x[b].T (fp32r matmuls, bias fused into DVE copies):
     Q^T, K^T in [d, t] layout (head-pair tiles), V in [t, d] layout with a
     ones column appended per head (softmax denominators come free out of
     the AV matmul).
  2. Flash-style attention in S^T = K Q^T layout (no transposes anywhere),
     processed in 512-wide q-chunks: S^T psum -> exp (ACT, 1/8 scale fused)
     -> causal mask on the diagonal 128-block (GpSimd mul) -> AV
     accumulation with [V | 1] as the stationary operand.  Normalization
     (row 64 = denominator) is batched: one [4, 512] reciprocal per chunk,
     partition_broadcast, GpSimd multiply.
  3. Per-chunk 4-core AllGather of y^T [256, 512] (pipelined with the next
     chunk's attention).
  4. Output projection column-sharded, emitted per chunk right behind its
     AllGather: each core computes o^T[e-slice, t-chunk] using its own
     w_proj slice (uniform SPMD program).  Host transposes + concatenates.
"""
import math

import numpy as np
import ml_dtypes

B, T, C, H = 2, 2048, 1024, 16
HD = C // H          # 64 head dim
G = 4                # head-groups (cores per batch)
HPG = H // G         # 4 heads per group
DG = HPG * HD        # 256 dims per group
N_CORES = 8
KC = C // 128        # 8 contraction chunks
NKT = T // 128       # 16 k-tiles
NQC = T // 512       # 4 q-chunks in attention
RG = [[0, 1, 2, 3], [4, 5, 6, 7]]

_NC_CACHE = {}


def _build():
    import concourse.bacc as bacc
    import concourse.mybir as mybir
    import concourse.tile as tile

    f32 = mybir.dt.float32
    f32r = mybir.dt.float32r
    bf16 = mybir.dt.bfloat16
    Exp = mybir.ActivationFunctionType.Exp

    nc = bacc.Bacc("TRN2", num_devices=N_CORES)

    xT_d = nc.dram_tensor("xT", [C, T], bf16, kind="ExternalInput")
    wq_d = nc.dram_tensor("wq", [C, DG], bf16, kind="ExternalInput")
    wk_d = nc.dram_tensor("wk", [C, DG], bf16, kind="ExternalInput")
    wv_d = nc.dram_tensor("wv", [C, DG], bf16, kind="ExternalInput")
    bq_d = nc.dram_tensor("bq", [2, 128, 1], f32, kind="ExternalInput")
    bk_d = nc.dram_tensor("bk", [2, 128, 1], f32, kind="ExternalInput")
    bv_d = nc.dram_tensor("bv", [1, DG], f32, kind="ExternalInput")
    # w_proj^T expanded over the 8-core AllGather row layout with the
    # foreign batch's rows zeroed host-side (keeps the program uniform)
    wp_d = nc.dram_tensor("wpTa", [C, DG], bf16, kind="ExternalInput")
    bp_d = nc.dram_tensor("bp", [2, 128, 1], f32, kind="ExternalInput")
    mask_d = nc.dram_tensor("mask", [128, 128], bf16, kind="ExternalInput")
    ones_d = nc.dram_tensor("ones4", [128, HPG, 1], bf16, kind="ExternalInput")
    oT_d = nc.dram_tensor("oT", [DG, T], f32, kind="ExternalOutput")

    def dma_chunked(dst, src, n):
        w = dst.shape[-1]
        step = w // n
        for i in range(n):
            nc.sync.dma_start(dst[..., step * i:step * (i + 1)],
                              src[..., step * i:step * (i + 1)])

    with tile.TileContext(nc) as tc:
        with (
            tc.tile_pool(name="persist", bufs=1) as persist,
            tc.tile_pool(name="dram", bufs=1, space="DRAM") as dram,
        ):
            # ---- persistent SBUF ----
            QT = [persist.tile([128, T], f32r, name=f"qt{p}") for p in range(2)]
            KT = [persist.tile([128, T], f32r, name=f"kt{p}") for p in range(2)]
            V1 = [persist.tile([128, HPG * (HD + 2)], bf16, name=f"v{m}")
                  for m in range(NKT)]
            wpT_sb = [persist.tile([128, DG], bf16, name=f"wp_{k}")
                      for k in range(KC)]
            mask_sb = persist.tile([128, 128], bf16, name="mask_sb")
            bq_sb = [persist.tile([128, 1], f32, name=f"bq{j}") for j in range(2)]
            bk_sb = [persist.tile([128, 1], f32, name=f"bk{j}") for j in range(2)]
            bp_sb = [persist.tile([128, 1], f32, name=f"bp{j}") for j in range(2)]
            bv_row = persist.tile([1, DG], f32, name="bv_row")
            bv_bc = persist.tile([128, DG], f32, name="bv_bc")

            # per-(chunk, pair) collective buffers (4-core ring AllGather);
            # chunks 1 and 0 share one merged buffer per pair (cols c1|c0)
            yq_in = [[dram.tile([128, 512], bf16, name=f"yqi{cq}_{p}")
                      for p in range(2)] for cq in (3, 2)]
            yq_out = [[dram.tile([512, 512], bf16, name=f"yqo{cq}_{p}")
                       for p in range(2)] for cq in (3, 2)]
            yqm_in = [dram.tile([128, 1024], bf16, name=f"yqmi{p}")
                      for p in range(2)]
            yqm_out = [dram.tile([512, 1024], bf16, name=f"yqmo{p}")
                       for p in range(2)]

            # ================= phase 1: QKV =================
            with (
                tc.tile_pool(name="xp", bufs=1) as xp,
                tc.tile_pool(name="wp_s", bufs=1) as wp_s,
                tc.tile_pool(name="qkvps", bufs=1, space="PSUM") as qkvps,
            ):
                # PE warmup: ~9us of dense dummy matmuls so HAM reaches
                # K=8/8 before the real (DMA-paced) matmuls arrive
                wu_a = xp.tile([128, 128], bf16, name="wu_a")
                wu_b = xp.tile([128, 512], bf16, name="wu_b")
                nc.vector.memset(wu_a[:], 0.5)
                nc.vector.memset(wu_b[:], 0.5)
                wu_ps = qkvps.tile([128, 512], f32, tag="qkvps", bufs=8,
                                   name="wu_ps")
                for _ in range(40):
                    nc.tensor.matmul(wu_ps[:], wu_a[:], wu_b[:],
                                     start=True, stop=True)

                # interleave wq/xT loads so the first Q matmul's inputs land
                # first (per-HWDGE-queue bandwidth is the startup limit)
                xT_sb = []
                wq_sb = []
                for k in range(KC):
                    wqt = wp_s.tile([128, DG], bf16, name=f"wq{k}")
                    dma_chunked(wqt, wq_d[128 * k:128 * (k + 1), :], 2)
                    wq_sb.append(wqt)
                    xt = xp.tile([128, T], bf16, name=f"x{k}")
                    dma_chunked(xt, xT_d[128 * k:128 * (k + 1), :], 4)
                    xT_sb.append(xt)

                nc.sync.dma_start(mask_sb[:], mask_d[:])
                for j in range(2):
                    nc.sync.dma_start(bq_sb[j][:], bq_d[j])
                    nc.sync.dma_start(bk_sb[j][:], bk_d[j])
                    nc.sync.dma_start(bp_sb[j][:], bp_d[j])
                nc.sync.dma_start(bv_row[:], bv_d[:])
                nc.gpsimd.partition_broadcast(bv_bc[:], bv_row[:])

                # Q then K: psum [2 jh][4 t4] accumulated over kc
                for sel in range(2):
                    dst = QT if sel == 0 else KT
                    bcol = bq_sb if sel == 0 else bk_sb
                    ps = [[qkvps.tile([128, 512], f32, tag="qkvps", bufs=8,
                                      name=f"ps{sel}_{jh}_{t4}")
                           for t4 in range(4)] for jh in range(2)]
                    for kc in range(KC):
                        if sel == 0:
                            wt = wq_sb[kc]
                        else:
                            wt = wp_s.tile([128, DG], bf16, tag="wqk", bufs=3,
                                           name=f"w{sel}_{kc}")
                            dma_chunked(wt, wk_d[128 * kc:128 * (kc + 1), :], 2)
                        for jh in range(2):
                            for t4 in range(4):
                                nc.tensor.matmul(
                                    ps[jh][t4][:],
                                    wt[:, 128 * jh:128 * (jh + 1)],
                                    xT_sb[kc][:, 512 * t4:512 * (t4 + 1)],
                                    start=(kc == 0), stop=(kc == KC - 1))
                    for jh in range(2):
                        for t4 in range(4):
                            nc.vector.tensor_scalar_add(
                                dst[jh][:, 512 * t4:512 * (t4 + 1)],
                                ps[jh][t4][:], bcol[jh][:])

                # V: [t, d] layout, heads at stride 65 with ones column
                wv_sb = []
                for k in range(KC):
                    wvt = wp_s.tile([128, DG], bf16, name=f"wv{k}")
                    dma_chunked(wvt, wv_d[128 * k:128 * (k + 1), :], 2)
                    wv_sb.append(wvt)
                for mt in range(NKT):
                    psv = qkvps.tile([128, DG], f32, tag="qkvps", bufs=8,
                                     name=f"psv{mt}")
                    for kc in range(KC):
                        nc.tensor.matmul(
                            psv[:],
                            xT_sb[kc][:, 128 * mt:128 * (mt + 1)],
                            wv_sb[kc][:],
                            start=(kc == 0), stop=(kc == KC - 1))
                    vv = V1[mt].rearrange("p (h x) -> p h x", h=HPG)
                    nc.vector.tensor_add(
                        vv[:, :, 0:HD],
                        psv.rearrange("p (h x) -> p h x", h=HPG),
                        bv_bc.rearrange("p (h x) -> p h x", h=HPG))
                    nc.sync.dma_start(vv[:, :, HD:HD + 1], ones_d[:])

            # ============ phase 2+3+4: attention / AG / projection ============
            for k in range(KC):
                nc.sync.dma_start(
                    wpT_sb[k][:], wp_d[128 * k:128 * (k + 1), :])

            with (
                tc.tile_pool(name="aps", bufs=1, space="PSUM") as aps,
                tc.tile_pool(name="ppool", bufs=1) as ppool,
                tc.tile_pool(name="npool", bufs=1) as npool,
                tc.tile_pool(name="ynp", bufs=1) as ynp,
                tc.tile_pool(name="yfp", bufs=1) as yfp,
                tc.tile_pool(name="otp", bufs=1) as otp,
            ):
                for cq in range(NQC - 1, -1, -1):
                    ycps = {}
                    for p in range(2):
                        rr_i = npool.tile([2, 512], f32, tag="rri", bufs=2,
                                          name=f"rri{cq}_{p}")
                        rr_o = npool.tile([2, 512], f32, tag="rro", bufs=2,
                                          name=f"rro{cq}_{p}")
                        yps = [aps.tile([HD + 1, 512], f32, tag=f"y{X}",
                                        bufs=1, name=f"y_{cq}_{p}_{X}")
                               for X in range(2)]
                        nkt = 4 * (cq + 1)
                        for kt in range(nkt):
                            qs = max(0, 128 * kt - 512 * cq)
                            qn = 512 - qs
                            S = aps.tile([128, 1024], f32, tag="s", bufs=2,
                                         name=f"s_{cq}_{p}_{kt}")
                            for X in range(2):
                                nc.tensor.matmul(
                                    S[:, 512 * X + qs:512 * (X + 1)],
                                    KT[p][64 * X:64 * (X + 1),
                                          128 * kt:128 * (kt + 1)],
                                    QT[p][64 * X:64 * (X + 1),
                                          512 * cq + qs:512 * (cq + 1)],
                                    start=True, stop=True)
                            Pt = ppool.tile([128, 1024], bf16, tag="p",
                                            bufs=4, name=f"p_{cq}_{p}_{kt}")
                            nc.scalar.activation(
                                out=Pt.rearrange("pp (x q) -> pp x q",
                                                 x=2)[:, :, qs:512],
                                in_=S.rearrange("pp (x q) -> pp x q",
                                                x=2)[:, :, qs:512],
                                func=Exp, scale=1.0 / math.sqrt(HD))
                            if kt >= 4 * cq:  # diagonal block: causal mask
                                for X in range(2):
                                    nc.vector.tensor_mul(
                                        Pt[:, 512 * X + qs:512 * X + qs + 128],
                                        Pt[:, 512 * X + qs:512 * X + qs + 128],
                                        mask_sb[:])
                            for X in range(2):
                                h = 2 * p + X
                                nc.tensor.matmul(
                                    yps[X][:, qs:512],
                                    V1[kt][:, (HD + 2) * h:
                                           (HD + 2) * h + HD + 1],
                                    Pt[:, 512 * X + qs:512 * (X + 1)],
                                    start=(kt == 0), stop=(kt == nkt - 1))
                        for X in range(2):
                            ycp = npool.tile([HD, 512], bf16, tag="ycp",
                                             bufs=4, name=f"yc_{cq}_{p}_{X}")
                            nc.vector.tensor_copy(ycp[:], yps[X][0:HD, :])
                            r1r = npool.tile([1, 512], f32, tag="r1r",
                                             bufs=4, name=f"r1r_{cq}_{p}_{X}")
                            nc.vector.tensor_copy(r1r[:], yps[X][HD:HD + 1, :])
                            nc.sync.dma_start(rr_i[X:X + 1, :], r1r[:])
                            ycps[(p, X)] = ycp
                        nc.vector.reciprocal(rr_o[:], rr_i[:])
                        yn = ynp.tile([128, 512], bf16, tag="yn", bufs=4,
                                      name=f"yn_{cq}_{p}")
                        for X in range(2):
                            r1 = npool.tile([1, 512], f32, tag="r1", bufs=4,
                                            name=f"r1_{cq}_{p}_{X}")
                            nc.sync.dma_start(r1[:], rr_o[X:X + 1, :])
                            bcx = npool.tile([HD, 512], f32, tag="bc",
                                             bufs=4, name=f"bcx_{cq}_{p}_{X}")
                            nc.gpsimd.partition_broadcast(bcx[:], r1[:])
                            nc.vector.tensor_mul(
                                yn[64 * X:64 * (X + 1), :],
                                ycps[(p, X)][:], bcx[:])
                        if cq >= 2:
                            dma_chunked(yq_in[3 - cq][p], yn, 2)
                            nc.gpsimd.collective_compute(
                                "AllGather", mybir.AluOpType.bypass,
                                replica_groups=RG,
                                ins=[yq_in[3 - cq][p][:].opt()],
                                outs=[yq_out[3 - cq][p][:].opt()],
                            )
                        else:
                            # cq 1 -> cols [0:512), cq 0 -> cols [512:1024)
                            co = 512 * (1 - cq)
                            dma_chunked(yqm_in[p][:, co:co + 512], yn, 2)
                            if cq == 0:
                                nc.gpsimd.collective_compute(
                                    "AllGather", mybir.AluOpType.bypass,
                                    replica_groups=RG,
                                    ins=[yqm_in[p][:].opt()],
                                    outs=[yqm_out[p][:].opt()],
                                )
                    if cq >= 2:
                        sub_list = [(yq_out[3 - cq], 0, cq)]
                    elif cq == 1:
                        sub_list = []
                    else:
                        sub_list = [(yqm_out, 0, 1), (yqm_out, 512, 0)]
                    for (buf, co, tq) in sub_list:
                        yfs = []
                        for kd in range(KC):
                            yf = yfp.tile([128, 512], bf16, tag="yf", bufs=8,
                                          name=f"yf_{tq}_{kd}")
                            g2, p2 = divmod(kd, 2)
                            dma_chunked(
                                yf,
                                buf[p2][128 * g2:128 * (g2 + 1), co:co + 512],
                                2)
                            yfs.append(yf)
                        for eh in range(2):
                            po = aps.tile([128, 512], f32, tag="po", bufs=2,
                                          name=f"po_{tq}_{eh}")
                            for kd in range(KC):
                                nc.tensor.matmul(
                                    po[:],
                                    wpT_sb[kd][:, 128 * eh:128 * (eh + 1)],
                                    yfs[kd][:],
                                    start=(kd == 0), stop=(kd == KC - 1))
                            ot = otp.tile([128, 512], f32, tag="ot", bufs=2,
                                          name=f"ot_{tq}_{eh}")
                            nc.vector.tensor_scalar_add(ot[:], po[:],
                                                        bp_sb[eh][:])
                            nc.sync.dma_start(
                                oT_d[128 * eh:128 * (eh + 1),
                                     512 * tq:512 * (tq + 1)], ot[:])

    nc.finalize()
    return nc


def _get_nc():
    if "nc" not in _NC_CACHE:
        _NC_CACHE["nc"] = _build()
    return _NC_CACHE["nc"]


def kernel(x, w_attn, b_attn, w_proj, b_proj):
    from concourse.bass_utils import run_bass_kernel_spmd

    x = np.asarray(x, dtype=np.float32)
    w_attn = np.asarray(w_attn, dtype=np.float32)
    b_attn = np.asarray(b_attn, dtype=np.float32)
    w_proj = np.asarray(w_proj, dtype=np.float32)
    b_proj = np.asarray(b_proj, dtype=np.float32)

    mask = np.triu(np.ones((128, 128), dtype=np.float32)).copy()

    in_maps = []
    for c in range(N_CORES):
        b, g = divmod(c, G)
        lo = DG * g
        wpT = np.ascontiguousarray(w_proj[lo:lo + DG, :].T)
        in_maps.append({
            "xT": np.ascontiguousarray(x[b].T).astype(ml_dtypes.bfloat16),
            "wq": np.ascontiguousarray(w_attn[lo:lo + DG, :].T).astype(ml_dtypes.bfloat16),
            "wk": np.ascontiguousarray(w_attn[C + lo:C + lo + DG, :].T).astype(ml_dtypes.bfloat16),
            "wv": np.ascontiguousarray(w_attn[2 * C + lo:2 * C + lo + DG, :].T).astype(ml_dtypes.bfloat16),
            "bq": np.ascontiguousarray(b_attn[lo:lo + DG].reshape(2, 128, 1)),
            "bk": np.ascontiguousarray(
                b_attn[C + lo:C + lo + DG].reshape(2, 128, 1)),
            "bv": np.ascontiguousarray(
                b_attn[2 * C + lo:2 * C + lo + DG].reshape(1, DG)),
            "wpTa": wpT.astype(ml_dtypes.bfloat16),
            "bp": np.ascontiguousarray(b_proj[lo:lo + DG].reshape(2, 128, 1)),
            "mask": mask.astype(ml_dtypes.bfloat16),
            "ones4": np.ones((128, HPG, 1), dtype=ml_dtypes.bfloat16),
        })

    global _last_in_maps
    _last_in_maps = in_maps

    nc = _get_nc()
    res = run_bass_kernel_spmd(nc, in_maps, list(range(N_CORES)))

    out = np.empty((B, T, C), dtype=np.float32)
    for c in range(N_CORES):
        b, g = divmod(c, G)
        out[b, :, DG * g:DG * (g + 1)] = res.results[c]["oT"].T
    return out



# revision 4
# speedup vs baseline: 1.2066x; 1.2066x over previous
"""Causal self-attention (B=2, T=2048, C=1024, H=16) on 8 trn2 NeuronCores.

Sharding: core c = (b, g) with b = c // 4 (batch), g = c % 4 (head-group of 4
heads = 256 dims). Per core:
  1. QKV projection from x[b].T (bf16 matmuls, bias fused into DVE evacs):
     Q^T, K^T in [d, t] bf16 layout (head-pair tiles), V in [t, d] layout with
     a ones column appended per head (softmax denominators come free out of
     the AV matmul).
  2. Flash-style attention in S^T = K Q^T layout (no transposes anywhere),
     processed in 512-wide q-chunks in INCREASING order: S^T psum -> exp
     (ACT, 1/8 scale fused) -> causal mask on the diagonal 128-blocks (DVE
     mul) -> AV accumulation with [V | 1] as the stationary operand.
     Normalization reads PSUM directly: per-head [1,512] denominator copy,
     reciprocal_approx_fast, partition_broadcast, multiply.
  3. 4-core AllGather of y^T per chunk, merged across the two head-pairs
     ([256, 512] in) for chunks 0-2; the last chunk keeps per-pair AGs so the
     first one overlaps pair 1's attention.
  4. Output projection column-sharded, proj(cq-1) emitted in the middle of
     chunk cq's attention so the PE never waits on a fresh AllGather.
     Host transposes + concatenates.
"""
import math

import numpy as np
import ml_dtypes

B, T, C, H = 2, 2048, 1024, 16
HD = C // H          # 64 head dim
G = 4                # head-groups (cores per batch)
HPG = H // G         # 4 heads per group
DG = HPG * HD        # 256 dims per group
N_CORES = 8
KC = C // 128        # 8 contraction chunks
NKT = T // 128       # 16 k-tiles
NQC = T // 512       # 4 q-chunks in attention
VS = HD + 2          # V head stride (64 dims + ones col + pad)
RG = [[0, 1, 2, 3], [4, 5, 6, 7]]

_NC_CACHE = {}


def _build():
    import concourse.bacc as bacc
    import concourse.mybir as mybir
    import concourse.tile as tile

    f32 = mybir.dt.float32
    bf16 = mybir.dt.bfloat16
    Exp = mybir.ActivationFunctionType.Exp

    nc = bacc.Bacc("TRN2", num_devices=N_CORES)

    xT_d = nc.dram_tensor("xT", [C, T], bf16, kind="ExternalInput")
    wq_d = nc.dram_tensor("wq", [C, DG], bf16, kind="ExternalInput")
    wk_d = nc.dram_tensor("wk", [C, DG], bf16, kind="ExternalInput")
    wv_d = nc.dram_tensor("wv", [C, DG], bf16, kind="ExternalInput")
    bq_d = nc.dram_tensor("bq", [2, 128, 1], f32, kind="ExternalInput")
    bk_d = nc.dram_tensor("bk", [2, 128, 1], f32, kind="ExternalInput")
    bv_d = nc.dram_tensor("bv", [1, DG], f32, kind="ExternalInput")
    wp_d = nc.dram_tensor("wpTa", [C, DG], bf16, kind="ExternalInput")
    bp_d = nc.dram_tensor("bp", [2, 128, 1], f32, kind="ExternalInput")
    mask_d = nc.dram_tensor("mask", [128, 128], bf16, kind="ExternalInput")
    ones_d = nc.dram_tensor("ones4", [128, HPG, 1], bf16, kind="ExternalInput")
    oT_d = nc.dram_tensor("oT", [DG, T], f32, kind="ExternalOutput")

    with tile.TileContext(nc) as tc:
        with (
            tc.tile_pool(name="persist", bufs=1) as persist,
            tc.tile_pool(name="dram", bufs=1, space="DRAM") as dram,
        ):
            # ---- persistent SBUF ----
            QT = [persist.tile([128, T], bf16, name=f"qt{p}") for p in range(2)]
            KT = [persist.tile([128, T], bf16, name=f"kt{p}") for p in range(2)]
            V1 = [persist.tile([128, HPG * VS], bf16, name=f"v{m}")
                  for m in range(NKT)]
            wpT_sb = [persist.tile([128, DG], bf16, name=f"wp_{k}")
                      for k in range(KC)]
            mask_sb = persist.tile([128, 128], bf16, name="mask_sb")
            bq_sb = [persist.tile([128, 1], f32, name=f"bq{j}") for j in range(2)]
            bk_sb = [persist.tile([128, 1], f32, name=f"bk{j}") for j in range(2)]
            bp_sb = [persist.tile([128, 1], f32, name=f"bp{j}") for j in range(2)]
            bv_row = persist.tile([1, DG], f32, name="bv_row")
            bv_bc = persist.tile([128, DG], f32, name="bv_bc")

            # collective DRAM buffers: merged-pair AGs for chunks 0-2,
            # per-pair AGs for the last chunk (tail overlap)
            yin_m = [dram.tile([256, 512], bf16, name=f"yim{cq}")
                     for cq in range(3)]
            yout_m = [dram.tile([1024, 512], bf16, name=f"yom{cq}")
                      for cq in range(3)]
            yin3 = [dram.tile([128, 512], bf16, name=f"yi3_{p}")
                    for p in range(2)]
            yout3 = [dram.tile([512, 512], bf16, name=f"yo3_{p}")
                     for p in range(2)]

            # ================= phase 1: QKV =================
            with (
                tc.tile_pool(name="xp", bufs=1) as xp,
                tc.tile_pool(name="wp_s", bufs=1) as wp_s,
                tc.tile_pool(name="qkvps", bufs=1, space="PSUM") as qkvps,
            ):
                # short PE warmup (pstate ramp) while the first loads land
                wu_a = xp.tile([128, 128], bf16, name="wu_a")
                wu_b = xp.tile([128, 512], bf16, name="wu_b")
                nc.vector.memset(wu_a[:], 0.5)
                nc.vector.memset(wu_b[:], 0.5)
                wu_ps = qkvps.tile([128, 512], f32, tag="qkvps", bufs=8,
                                   name="wu_ps")
                for _ in range(8):
                    nc.tensor.matmul(wu_ps[:], wu_a[:], wu_b[:],
                                     start=True, stop=True)

                # input loads: weights on the sync queue; x tiles split
                # across the scalar/vector queues so three HWDGE rings run
                # in parallel (per-ring bandwidth is the startup limit)
                xT_sb = []
                wq_sb = []
                wk_sb = []
                wv_sb = []
                for k in range(KC):
                    wqt = wp_s.tile([128, DG], bf16, name=f"wq{k}")
                    nc.sync.dma_start(wqt[:], wq_d[128 * k:128 * (k + 1), :])
                    wq_sb.append(wqt)
                    xt = xp.tile([128, T], bf16, name=f"x{k}")
                    eng = nc.scalar if k % 2 == 0 else nc.gpsimd
                    eng.dma_start(xt[:, 0:1024], xT_d[128 * k:128 * (k + 1),
                                                      0:1024])
                    eng.dma_start(xt[:, 1024:2048], xT_d[128 * k:128 * (k + 1),
                                                         1024:2048])
                    xT_sb.append(xt)
                for k in range(KC):
                    wkt = wp_s.tile([128, DG], bf16, name=f"wk{k}")
                    nc.sync.dma_start(wkt[:], wk_d[128 * k:128 * (k + 1), :])
                    wk_sb.append(wkt)
                    wvt = wp_s.tile([128, DG], bf16, name=f"wv{k}")
                    nc.sync.dma_start(wvt[:], wv_d[128 * k:128 * (k + 1), :])
                    wv_sb.append(wvt)

                nc.sync.dma_start(mask_sb[:], mask_d[:])
                for j in range(2):
                    nc.sync.dma_start(bq_sb[j][:], bq_d[j])
                    nc.sync.dma_start(bk_sb[j][:], bk_d[j])
                    nc.sync.dma_start(bp_sb[j][:], bp_d[j])
                nc.sync.dma_start(bv_row[:], bv_d[:])
                nc.gpsimd.partition_broadcast(bv_bc[:], bv_row[:])

                # Q then K: psum [2 jh][4 t4] accumulated over kc
                for sel in range(2):
                    dst = QT if sel == 0 else KT
                    wsb = wq_sb if sel == 0 else wk_sb
                    bcol = bq_sb if sel == 0 else bk_sb
                    ps = [[qkvps.tile([128, 512], f32, tag="qkvps", bufs=8,
                                      name=f"ps{sel}_{jh}_{t4}")
                           for t4 in range(4)] for jh in range(2)]
                    for kc in range(KC):
                        for jh in range(2):
                            for t4 in range(4):
                                nc.tensor.matmul(
                                    ps[jh][t4][:],
                                    wsb[kc][:, 128 * jh:128 * (jh + 1)],
                                    xT_sb[kc][:, 512 * t4:512 * (t4 + 1)],
                                    start=(kc == 0), stop=(kc == KC - 1))
                    for jh in range(2):
                        for t4 in range(4):
                            nc.vector.tensor_scalar_add(
                                dst[jh][:, 512 * t4:512 * (t4 + 1)],
                                ps[jh][t4][:], bcol[jh][:])

                # V: [t, d] layout, heads at stride VS with ones column
                for mt in range(NKT):
                    psv = qkvps.tile([128, DG], f32, tag="qkvps", bufs=8,
                                     name=f"psv{mt}")
                    for kc in range(KC):
                        nc.tensor.matmul(
                            psv[:],
                            xT_sb[kc][:, 128 * mt:128 * (mt + 1)],
                            wv_sb[kc][:],
                            start=(kc == 0), stop=(kc == KC - 1))
                    vv = V1[mt].rearrange("p (h x) -> p h x", h=HPG)
                    nc.vector.tensor_add(
                        vv[:, :, 0:HD],
                        psv.rearrange("p (h x) -> p h x", h=HPG),
                        bv_bc.rearrange("p (h x) -> p h x", h=HPG))
                    nc.sync.dma_start(vv[:, :, HD:HD + 1], ones_d[:])

            # ============ phase 2: attention / AG / projection ============
            for k in range(KC):
                nc.sync.dma_start(
                    wpT_sb[k][:], wp_d[128 * k:128 * (k + 1), :])

            with (
                tc.tile_pool(name="aps", bufs=1, space="PSUM") as aps,
                tc.tile_pool(name="ppool", bufs=1) as ppool,
                tc.tile_pool(name="npool", bufs=1) as npool,
                tc.tile_pool(name="ynp", bufs=1) as ynp,
                tc.tile_pool(name="yfp", bufs=1) as yfp,
                tc.tile_pool(name="otp", bufs=1) as otp,
            ):
                def emit_attn_pair(cq, p):
                    """S^T -> exp -> mask -> AV -> normalize -> yn tile."""
                    yps = [aps.tile([HD + 1, 512], f32, tag=f"y{X}",
                                    bufs=1, name=f"y_{cq}_{p}_{X}")
                           for X in range(2)]
                    nkt = 4 * (cq + 1)
                    for kt in range(nkt):
                        qs = max(0, 128 * kt - 512 * cq)
                        S = aps.tile([128, 1024], f32, tag="s", bufs=2,
                                     name=f"s_{cq}_{p}_{kt}")
                        for X in range(2):
                            nc.tensor.matmul(
                                S[:, 512 * X + qs:512 * (X + 1)],
                                KT[p][64 * X:64 * (X + 1),
                                      128 * kt:128 * (kt + 1)],
                                QT[p][64 * X:64 * (X + 1),
                                      512 * cq + qs:512 * (cq + 1)],
                                start=True, stop=True)
                        Pt = ppool.tile([128, 1024], bf16, tag="p",
                                        bufs=4, name=f"p_{cq}_{p}_{kt}")
                        nc.scalar.activation(
                            out=Pt.rearrange("pp (x q) -> pp x q",
                                             x=2)[:, :, qs:512],
                            in_=S.rearrange("pp (x q) -> pp x q",
                                            x=2)[:, :, qs:512],
                            func=Exp, scale=1.0 / math.sqrt(HD))
                        if kt >= 4 * cq:  # diagonal block: causal mask
                            for X in range(2):
                                nc.vector.tensor_mul(
                                    Pt[:, 512 * X + qs:512 * X + qs + 128],
                                    Pt[:, 512 * X + qs:512 * X + qs + 128],
                                    mask_sb[:])
                        for X in range(2):
                            h = 2 * p + X
                            nc.tensor.matmul(
                                yps[X][:, qs:512],
                                V1[kt][:, VS * h:VS * h + HD + 1],
                                Pt[:, 512 * X + qs:512 * (X + 1)],
                                start=(kt == 0), stop=(kt == nkt - 1))
                    # normalization: denominator row 64 -> 1/x -> broadcast
                    yn = ynp.tile([128, 512], bf16, tag="yn", bufs=4,
                                  name=f"yn_{cq}_{p}")
                    for X in range(2):
                        r1r = npool.tile([1, 512], f32, tag="r1r",
                                         bufs=4, name=f"r1r_{cq}_{p}_{X}")
                        nc.vector.tensor_copy(r1r[:], yps[X][HD:HD + 1, :])
                        rro = npool.tile([1, 512], f32, tag="rro",
                                         bufs=4, name=f"rro_{cq}_{p}_{X}")
                        nc.vector.reciprocal_approx_fast(rro[:], r1r[:])
                        bcx = npool.tile([HD, 512], f32, tag="bc",
                                         bufs=4, name=f"bcx_{cq}_{p}_{X}")
                        nc.gpsimd.partition_broadcast(bcx[:], rro[:])
                        nc.vector.tensor_mul(
                            yn[64 * X:64 * (X + 1), :],
                            yps[X][0:HD, :], bcx[:])
                    return yn

                def emit_proj(tq):
                    """o^T[:, 512tq:512(tq+1)] from the AllGathered y^T."""
                    if tq == 3:
                        order = [0, 2, 4, 6, 1, 3, 5, 7]
                    else:
                        order = list(range(KC))
                    yfs = {}
                    for kd in order:
                        yf = yfp.tile([128, 512], bf16, tag="yf", bufs=8,
                                      name=f"yf_{tq}_{kd}")
                        g2, p2 = divmod(kd, 2)
                        if tq == 3:
                            src = yout3[p2][128 * g2:128 * (g2 + 1), :]
                        else:
                            src = yout_m[tq][256 * g2 + 128 * p2:
                                             256 * g2 + 128 * p2 + 128, :]
                        nc.scalar.dma_start(yf[:], src)
                        yfs[kd] = yf
                    po = [aps.tile([128, 512], f32, tag="po", bufs=2,
                                   name=f"po_{tq}_{eh}") for eh in range(2)]
                    for i, kd in enumerate(order):
                        for eh in range(2):
                            nc.tensor.matmul(
                                po[eh][:],
                                wpT_sb[kd][:, 128 * eh:128 * (eh + 1)],
                                yfs[kd][:],
                                start=(i == 0), stop=(i == KC - 1))
                    for eh in range(2):
                        ot = otp.tile([128, 512], f32, tag="ot", bufs=2,
                                      name=f"ot_{tq}_{eh}")
                        nc.vector.tensor_scalar_add(ot[:], po[eh][:],
                                                    bp_sb[eh][:])
                        nc.sync.dma_start(
                            oT_d[128 * eh:128 * (eh + 1),
                                 512 * tq:512 * (tq + 1)], ot[:])

                for cq in range(NQC):
                    last = cq == NQC - 1
                    for p in range(2):
                        yn = emit_attn_pair(cq, p)
                        if last:
                            nc.sync.dma_start(yin3[p][:], yn[:])
                            nc.gpsimd.collective_compute(
                                "AllGather", mybir.AluOpType.bypass,
                                replica_groups=RG,
                                ins=[yin3[p][:].opt()],
                                outs=[yout3[p][:].opt()],
                            )
                        else:
                            nc.sync.dma_start(
                                yin_m[cq][128 * p:128 * (p + 1), :], yn[:])
                        if p == 0 and cq >= 1:
                            emit_proj(cq - 1)
                    if not last:
                        nc.gpsimd.collective_compute(
                            "AllGather", mybir.AluOpType.bypass,
                            replica_groups=RG,
                            ins=[yin_m[cq][:].opt()],
                            outs=[yout_m[cq][:].opt()],
                        )
                emit_proj(3)

    nc.finalize()
    return nc


def _get_nc():
    if "nc" not in _NC_CACHE:
        _NC_CACHE["nc"] = _build()
    return _NC_CACHE["nc"]


def kernel(x, w_attn, b_attn, w_proj, b_proj):
    from concourse.bass_utils import run_bass_kernel_spmd

    x = np.asarray(x, dtype=np.float32)
    w_attn = np.asarray(w_attn, dtype=np.float32)
    b_attn = np.asarray(b_attn, dtype=np.float32)
    w_proj = np.asarray(w_proj, dtype=np.float32)
    b_proj = np.asarray(b_proj, dtype=np.float32)

    mask = np.triu(np.ones((128, 128), dtype=np.float32)).copy()

    in_maps = []
    for c in range(N_CORES):
        b, g = divmod(c, G)
        lo = DG * g
        wpT = np.ascontiguousarray(w_proj[lo:lo + DG, :].T)
        in_maps.append({
            "xT": np.ascontiguousarray(x[b].T).astype(ml_dtypes.bfloat16),
            "wq": np.ascontiguousarray(w_attn[lo:lo + DG, :].T).astype(ml_dtypes.bfloat16),
            "wk": np.ascontiguousarray(w_attn[C + lo:C + lo + DG, :].T).astype(ml_dtypes.bfloat16),
            "wv": np.ascontiguousarray(w_attn[2 * C + lo:2 * C + lo + DG, :].T).astype(ml_dtypes.bfloat16),
            "bq": np.ascontiguousarray(b_attn[lo:lo + DG].reshape(2, 128, 1)),
            "bk": np.ascontiguousarray(
                b_attn[C + lo:C + lo + DG].reshape(2, 128, 1)),
            "bv": np.ascontiguousarray(
                b_attn[2 * C + lo:2 * C + lo + DG].reshape(1, DG)),
            "wpTa": wpT.astype(ml_dtypes.bfloat16),
            "bp": np.ascontiguousarray(b_proj[lo:lo + DG].reshape(2, 128, 1)),
            "mask": mask.astype(ml_dtypes.bfloat16),
            "ones4": np.ones((128, HPG, 1), dtype=ml_dtypes.bfloat16),
        })

    global _last_in_maps
    _last_in_maps = in_maps

    nc = _get_nc()
    res = run_bass_kernel_spmd(nc, in_maps, list(range(N_CORES)))

    out = np.empty((B, T, C), dtype=np.float32)
    for c in range(N_CORES):
        b, g = divmod(c, G)
        out[b, :, DG * g:DG * (g + 1)] = res.results[c]["oT"].T
    return out


# revision 9
# speedup vs baseline: 1.2965x; 1.0745x over previous
"""Causal self-attention (B=2, T=2048, C=1024, H=16) on 8 trn2 NeuronCores.

Sharding: core c = (b, g) with b = c // 4 (batch), g = c % 4 (head-group of 4
heads = 256 dims). Per core:
  1. QKV projection from x[b].T (bf16 matmuls, bias fused into DVE evacs):
     Q^T, K^T in [d, t] bf16 layout (head-pair tiles), V in [t, d] layout with
     a ones column appended per head (softmax denominators come free out of
     the AV matmul).
  2. Flash-style attention in S^T = K Q^T layout (no transposes anywhere),
     processed in 512-wide q-chunks in INCREASING order: S^T psum -> exp
     (ACT, 1/8 scale fused) -> causal mask on the diagonal 128-blocks (DVE
     mul) -> AV accumulation with [V | 1] as the stationary operand.
     Normalization reads PSUM directly: per-head [1,512] denominator copy,
     reciprocal_approx_fast, partition_broadcast, multiply.
  3. 4-core AllGather of y^T per chunk, merged across the two head-pairs
     ([256, 512] in) for chunks 0-2; the last chunk keeps per-pair AGs so the
     first one overlaps pair 1's attention.
  4. Output projection column-sharded, proj(cq-1) emitted in the middle of
     chunk cq's attention so the PE never waits on a fresh AllGather.
     Host transposes + concatenates.
"""
import math

import numpy as np
import ml_dtypes

B, T, C, H = 2, 2048, 1024, 16
HD = C // H          # 64 head dim
G = 4                # head-groups (cores per batch)
HPG = H // G         # 4 heads per group
DG = HPG * HD        # 256 dims per group
N_CORES = 8
KC = C // 128        # 8 contraction chunks
NKT = T // 128       # 16 k-tiles
NQC = T // 512       # 4 q-chunks in attention
VS = HD + 2          # V head stride (64 dims + ones col + pad)
RG = [[0, 1, 2, 3], [4, 5, 6, 7]]

_NC_CACHE = {}


def _build():
    import concourse.bacc as bacc
    import concourse.mybir as mybir
    import concourse.tile as tile

    f32 = mybir.dt.float32
    bf16 = mybir.dt.bfloat16
    Exp = mybir.ActivationFunctionType.Exp

    nc = bacc.Bacc("TRN2", num_devices=N_CORES)

    xT_d = nc.dram_tensor("xT", [C, T], bf16, kind="ExternalInput")
    wq_d = nc.dram_tensor("wq", [C, DG], bf16, kind="ExternalInput")
    wk_d = nc.dram_tensor("wk", [C, DG], bf16, kind="ExternalInput")
    wv_d = nc.dram_tensor("wv", [C, DG], bf16, kind="ExternalInput")
    bq_d = nc.dram_tensor("bq", [2, 128, 1], f32, kind="ExternalInput")
    bk_d = nc.dram_tensor("bk", [2, 128, 1], f32, kind="ExternalInput")
    bv_d = nc.dram_tensor("bv", [1, DG], f32, kind="ExternalInput")
    wp_d = nc.dram_tensor("wpTa", [C, DG], bf16, kind="ExternalInput")
    bp_d = nc.dram_tensor("bp", [2, 128, 1], f32, kind="ExternalInput")
    mask_d = nc.dram_tensor("mask", [128, 128], bf16, kind="ExternalInput")
    ones_d = nc.dram_tensor("ones4", [128, HPG, 1], bf16, kind="ExternalInput")
    oT_d = nc.dram_tensor("oT", [DG, T], f32, kind="ExternalOutput")

    with tile.TileContext(nc) as tc:
        with (
            tc.tile_pool(name="persist", bufs=1) as persist,
            tc.tile_pool(name="dram", bufs=1, space="DRAM") as dram,
        ):
            # ---- persistent SBUF ----
            QT = [persist.tile([128, T], bf16, name=f"qt{p}") for p in range(2)]
            KT = [persist.tile([128, T], bf16, name=f"kt{p}") for p in range(2)]
            V1 = [persist.tile([128, HPG * VS], bf16, name=f"v{m}")
                  for m in range(NKT)]
            wpT_sb = [persist.tile([128, DG], bf16, name=f"wp_{k}")
                      for k in range(KC)]
            mask_sb = persist.tile([128, 128], bf16, name="mask_sb")
            bq_sb = [persist.tile([128, 1], f32, name=f"bq{j}") for j in range(2)]
            bk_sb = [persist.tile([128, 1], f32, name=f"bk{j}") for j in range(2)]
            bp_sb = [persist.tile([128, 1], f32, name=f"bp{j}") for j in range(2)]
            bv_row = persist.tile([1, DG], f32, name="bv_row")
            bv_bc = persist.tile([128, DG], f32, name="bv_bc")

            # collective DRAM buffers: merged-pair AGs for chunks 0-2,
            # per-pair AGs for the last chunk (tail overlap)
            yin_m = [dram.tile([256, 512], bf16, name=f"yim{cq}")
                     for cq in range(3)]
            yout_m = [dram.tile([1024, 512], bf16, name=f"yom{cq}")
                      for cq in range(3)]
            yin3 = [dram.tile([128, 512], bf16, name=f"yi3_{p}")
                    for p in range(2)]
            yout3 = [dram.tile([512, 512], bf16, name=f"yo3_{p}")
                     for p in range(2)]
            prime_in = dram.tile([128, 16], bf16, name="prime_in")
            prime_out = dram.tile([512, 16], bf16, name="prime_out")

            # ================= phase 1: QKV =================
            with (
                tc.tile_pool(name="xp", bufs=1) as xp,
                tc.tile_pool(name="wp_s", bufs=1) as wp_s,
                tc.tile_pool(name="qkvps", bufs=1, space="PSUM") as qkvps,
            ):
                # short PE warmup (pstate ramp) while the first loads land
                wu_a = xp.tile([128, 128], bf16, name="wu_a")
                wu_b = xp.tile([128, 512], bf16, name="wu_b")
                nc.vector.memset(wu_a[:], 0.5)
                nc.vector.memset(wu_b[:], 0.5)
                wu_ps = qkvps.tile([128, 512], f32, tag="qkvps", bufs=8,
                                   name="wu_ps")
                for _ in range(8):
                    nc.tensor.matmul(wu_ps[:], wu_a[:], wu_b[:],
                                     start=True, stop=True)

                # input loads: weights on the sync queue; x tiles split
                # across the scalar/vector queues so three HWDGE rings run
                # in parallel (per-ring bandwidth is the startup limit)
                xT_sb = []
                wq_sb = []
                wk_sb = []
                wv_sb = []
                for k in range(KC):
                    wqt = wp_s.tile([128, DG], bf16, name=f"wq{k}")
                    nc.sync.dma_start(wqt[:], wq_d[128 * k:128 * (k + 1), :])
                    wq_sb.append(wqt)
                    xt = xp.tile([128, T], bf16, name=f"x{k}")
                    eng = nc.scalar if k % 2 == 0 else nc.gpsimd
                    eng.dma_start(xt[:, 0:1024], xT_d[128 * k:128 * (k + 1),
                                                      0:1024])
                    eng.dma_start(xt[:, 1024:2048], xT_d[128 * k:128 * (k + 1),
                                                         1024:2048])
                    xT_sb.append(xt)
                for k in range(KC):
                    wkt = wp_s.tile([128, DG], bf16, name=f"wk{k}")
                    nc.sync.dma_start(wkt[:], wk_d[128 * k:128 * (k + 1), :])
                    wk_sb.append(wkt)
                    wvt = wp_s.tile([128, DG], bf16, name=f"wv{k}")
                    nc.sync.dma_start(wvt[:], wv_d[128 * k:128 * (k + 1), :])
                    wv_sb.append(wvt)

                nc.sync.dma_start(mask_sb[:], mask_d[:])
                for j in range(2):
                    nc.sync.dma_start(bq_sb[j][:], bq_d[j])
                    nc.sync.dma_start(bk_sb[j][:], bk_d[j])
                    nc.sync.dma_start(bp_sb[j][:], bp_d[j])
                nc.sync.dma_start(bv_row[:], bv_d[:])
                nc.gpsimd.partition_broadcast(bv_bc[:], bv_row[:])
                # priming collective: absorbs the first-op rendezvous skew /
                # CC-stream startup cost while QKV runs, so AG(0) is fast
                nc.gpsimd.collective_compute(
                    "AllGather", mybir.AluOpType.bypass,
                    replica_groups=RG,
                    ins=[prime_in[:].opt()],
                    outs=[prime_out[:].opt()],
                )

                # Q then K: psum [2 jh][4 t4] accumulated over kc
                for sel in range(2):
                    dst = QT if sel == 0 else KT
                    wsb = wq_sb if sel == 0 else wk_sb
                    bcol = bq_sb if sel == 0 else bk_sb
                    ps = [[qkvps.tile([128, 512], f32, tag="qkvps", bufs=8,
                                      name=f"ps{sel}_{jh}_{t4}")
                           for t4 in range(4)] for jh in range(2)]
                    for kc in range(KC):
                        for jh in range(2):
                            for t4 in range(4):
                                nc.tensor.matmul(
                                    ps[jh][t4][:],
                                    wsb[kc][:, 128 * jh:128 * (jh + 1)],
                                    xT_sb[kc][:, 512 * t4:512 * (t4 + 1)],
                                    start=(kc == 0), stop=(kc == KC - 1))
                    for jh in range(2):
                        for t4 in range(4):
                            nc.vector.tensor_scalar_add(
                                dst[jh][:, 512 * t4:512 * (t4 + 1)],
                                ps[jh][t4][:], bcol[jh][:])

                # V: [t, d] layout, heads at stride VS with ones column
                for mt in range(NKT):
                    psv = qkvps.tile([128, DG], f32, tag="qkvps", bufs=8,
                                     name=f"psv{mt}")
                    for kc in range(KC):
                        nc.tensor.matmul(
                            psv[:],
                            xT_sb[kc][:, 128 * mt:128 * (mt + 1)],
                            wv_sb[kc][:],
                            start=(kc == 0), stop=(kc == KC - 1))
                    vv = V1[mt].rearrange("p (h x) -> p h x", h=HPG)
                    nc.vector.tensor_add(
                        vv[:, :, 0:HD],
                        psv.rearrange("p (h x) -> p h x", h=HPG),
                        bv_bc.rearrange("p (h x) -> p h x", h=HPG))
                    nc.sync.dma_start(vv[:, :, HD:HD + 1], ones_d[:])

            # ============ phase 2: attention / AG / projection ============
            for k in range(KC):
                nc.sync.dma_start(
                    wpT_sb[k][:], wp_d[128 * k:128 * (k + 1), :])

            with (
                tc.tile_pool(name="aps", bufs=1, space="PSUM") as aps,
                tc.tile_pool(name="ppool", bufs=1) as ppool,
                tc.tile_pool(name="npool", bufs=1) as npool,
                tc.tile_pool(name="ynp", bufs=1) as ynp,
                tc.tile_pool(name="yfp", bufs=1) as yfp,
                tc.tile_pool(name="otp", bufs=1) as otp,
            ):
                def emit_attn_pair(cq, p, mid_cb=None):
                    """S^T -> exp -> mask -> AV -> normalize -> yn tile."""
                    yps = [aps.tile([HD + 1, 512], f32, tag=f"y{X}",
                                    bufs=1, name=f"y_{cq}_{p}_{X}")
                           for X in range(2)]
                    nkt = 4 * (cq + 1)
                    for kt in range(nkt):
                        if kt == 8 and mid_cb is not None:
                            mid_cb()
                        qs = max(0, 128 * kt - 512 * cq)
                        S = aps.tile([128, 1024], f32, tag="s", bufs=2,
                                     name=f"s_{cq}_{p}_{kt}")
                        for X in range(2):
                            nc.tensor.matmul(
                                S[:, 512 * X + qs:512 * (X + 1)],
                                KT[p][64 * X:64 * (X + 1),
                                      128 * kt:128 * (kt + 1)],
                                QT[p][64 * X:64 * (X + 1),
                                      512 * cq + qs:512 * (cq + 1)],
                                start=True, stop=True)
                        Pt = ppool.tile([128, 1024], bf16, tag="p",
                                        bufs=4, name=f"p_{cq}_{p}_{kt}")
                        nc.scalar.activation(
                            out=Pt.rearrange("pp (x q) -> pp x q",
                                             x=2)[:, :, qs:512],
                            in_=S.rearrange("pp (x q) -> pp x q",
                                            x=2)[:, :, qs:512],
                            func=Exp, scale=1.0 / math.sqrt(HD))
                        if kt >= 4 * cq:  # diagonal block: causal mask
                            for X in range(2):
                                nc.vector.tensor_mul(
                                    Pt[:, 512 * X + qs:512 * X + qs + 128],
                                    Pt[:, 512 * X + qs:512 * X + qs + 128],
                                    mask_sb[:])
                        for X in range(2):
                            h = 2 * p + X
                            nc.tensor.matmul(
                                yps[X][:, qs:512],
                                V1[kt][:, VS * h:VS * h + HD + 1],
                                Pt[:, 512 * X + qs:512 * (X + 1)],
                                start=(kt == 0), stop=(kt == nkt - 1))
                    # normalization: denominator row 64 -> 1/x -> broadcast
                    yn = ynp.tile([128, 512], bf16, tag="yn", bufs=4,
                                  name=f"yn_{cq}_{p}")
                    for X in range(2):
                        r1r = npool.tile([1, 512], f32, tag="r1r",
                                         bufs=4, name=f"r1r_{cq}_{p}_{X}")
                        nc.vector.tensor_copy(r1r[:], yps[X][HD:HD + 1, :])
                        rro = npool.tile([1, 512], f32, tag="rro",
                                         bufs=4, name=f"rro_{cq}_{p}_{X}")
                        nc.vector.reciprocal_approx_fast(rro[:], r1r[:])
                        bcx = npool.tile([HD, 512], f32, tag="bc",
                                         bufs=4, name=f"bcx_{cq}_{p}_{X}")
                        nc.gpsimd.partition_broadcast(bcx[:], rro[:])
                        nc.vector.tensor_mul(
                            yn[64 * X:64 * (X + 1), :],
                            yps[X][0:HD, :], bcx[:])
                    return yn

                def emit_proj(tq):
                    """o^T[:, 512tq:512(tq+1)] from the AllGathered y^T."""
                    if tq == 3:
                        order = [0, 2, 4, 6, 1, 3, 5, 7]
                    else:
                        order = list(range(KC))
                    yfs = {}
                    for kd in order:
                        yf = yfp.tile([128, 512], bf16, tag="yf", bufs=8,
                                      name=f"yf_{tq}_{kd}")
                        g2, p2 = divmod(kd, 2)
                        if tq == 3:
                            src = yout3[p2][128 * g2:128 * (g2 + 1), :]
                        else:
                            src = yout_m[tq][256 * g2 + 128 * p2:
                                             256 * g2 + 128 * p2 + 128, :]
                        nc.sync.dma_start(yf[:], src)
                        yfs[kd] = yf
                    po = [aps.tile([128, 512], f32, tag="po", bufs=2,
                                   name=f"po_{tq}_{eh}") for eh in range(2)]
                    for i, kd in enumerate(order):
                        for eh in range(2):
                            nc.tensor.matmul(
                                po[eh][:],
                                wpT_sb[kd][:, 128 * eh:128 * (eh + 1)],
                                yfs[kd][:],
                                start=(i == 0), stop=(i == KC - 1))
                    for eh in range(2):
                        ot = otp.tile([128, 512], f32, tag="ot", bufs=2,
                                      name=f"ot_{tq}_{eh}")
                        nc.vector.tensor_scalar_add(ot[:], po[eh][:],
                                                    bp_sb[eh][:])
                        nc.sync.dma_start(
                            oT_d[128 * eh:128 * (eh + 1),
                                 512 * tq:512 * (tq + 1)], ot[:])

                for cq in range(NQC):
                    last = cq == NQC - 1
                    for p in range(2):
                        # proj(cq-2) fills the PE between the two pairs;
                        # proj(2) goes mid-way into the last pair so AG(2)
                        # has time to land
                        mid = (lambda: emit_proj(2)) if (last and p == 1) \
                            else None
                        yn = emit_attn_pair(cq, p, mid_cb=mid)
                        if last:
                            nc.sync.dma_start(yin3[p][:], yn[:])
                            nc.gpsimd.collective_compute(
                                "AllGather", mybir.AluOpType.bypass,
                                replica_groups=RG,
                                ins=[yin3[p][:].opt()],
                                outs=[yout3[p][:].opt()],
                            )
                        else:
                            nc.sync.dma_start(
                                yin_m[cq][128 * p:128 * (p + 1), :], yn[:])
                        if p == 0 and cq >= 2:
                            emit_proj(cq - 2)
                    if not last:
                        nc.gpsimd.collective_compute(
                            "AllGather", mybir.AluOpType.bypass,
                            replica_groups=RG,
                            ins=[yin_m[cq][:].opt()],
                            outs=[yout_m[cq][:].opt()],
                        )
                emit_proj(3)

    nc.finalize()
    return nc


def _get_nc():
    if "nc" not in _NC_CACHE:
        _NC_CACHE["nc"] = _build()
    return _NC_CACHE["nc"]


def kernel(x, w_attn, b_attn, w_proj, b_proj):
    from concourse.bass_utils import run_bass_kernel_spmd

    x = np.asarray(x, dtype=np.float32)
    w_attn = np.asarray(w_attn, dtype=np.float32)
    b_attn = np.asarray(b_attn, dtype=np.float32)
    w_proj = np.asarray(w_proj, dtype=np.float32)
    b_proj = np.asarray(b_proj, dtype=np.float32)

    mask = np.triu(np.ones((128, 128), dtype=np.float32)).copy()

    in_maps = []
    for c in range(N_CORES):
        b, g = divmod(c, G)
        lo = DG * g
        wpT = np.ascontiguousarray(w_proj[lo:lo + DG, :].T)
        in_maps.append({
            "xT": np.ascontiguousarray(x[b].T).astype(ml_dtypes.bfloat16),
            "wq": np.ascontiguousarray(w_attn[lo:lo + DG, :].T).astype(ml_dtypes.bfloat16),
            "wk": np.ascontiguousarray(w_attn[C + lo:C + lo + DG, :].T).astype(ml_dtypes.bfloat16),
            "wv": np.ascontiguousarray(w_attn[2 * C + lo:2 * C + lo + DG, :].T).astype(ml_dtypes.bfloat16),
            "bq": np.ascontiguousarray(b_attn[lo:lo + DG].reshape(2, 128, 1)),
            "bk": np.ascontiguousarray(
                b_attn[C + lo:C + lo + DG].reshape(2, 128, 1)),
            "bv": np.ascontiguousarray(
                b_attn[2 * C + lo:2 * C + lo + DG].reshape(1, DG)),
            "wpTa": wpT.astype(ml_dtypes.bfloat16),
            "bp": np.ascontiguousarray(b_proj[lo:lo + DG].reshape(2, 128, 1)),
            "mask": mask.astype(ml_dtypes.bfloat16),
            "ones4": np.ones((128, HPG, 1), dtype=ml_dtypes.bfloat16),
        })

    global _last_in_maps
    _last_in_maps = in_maps

    nc = _get_nc()
    res = run_bass_kernel_spmd(nc, in_maps, list(range(N_CORES)))

    out = np.empty((B, T, C), dtype=np.float32)
    for c in range(N_CORES):
        b, g = divmod(c, G)
        out[b, :, DG * g:DG * (g + 1)] = res.results[c]["oT"].T
    return out


# revision 16
# speedup vs baseline: 1.3576x; 1.0471x over previous
"""Causal self-attention (B=2, T=2048, C=1024, H=16) on 8 trn2 NeuronCores.

Sharding: core c = (b, g) with b = c // 4 (batch), g = c % 4 (head-group of 4
heads = 256 dims). Per core:
  1. QKV projection from x[b].T (bf16 matmuls, bias fused into DVE evacs):
     Q^T, K^T in [d, t] bf16 layout (head-pair tiles), V in [t, d] layout with
     a ones column appended per head (softmax denominators come free out of
     the AV matmul).
  2. Flash-style attention in S^T = K Q^T layout (no transposes anywhere),
     processed in 512-wide q-chunks in INCREASING order: S^T psum -> exp
     (ACT, 1/8 scale fused) -> causal mask on the diagonal 128-blocks (DVE
     mul) -> AV accumulation with [V | 1] as the stationary operand.
     Normalization reads PSUM directly: per-head [1,512] denominator copy,
     reciprocal_approx_fast, partition_broadcast, multiply.
  3. 4-core AllGather of y^T per chunk, merged across the two head-pairs
     ([256, 512] in) for chunks 0-2; the last chunk keeps per-pair AGs so the
     first one overlaps pair 1's attention.
  4. Output projection column-sharded, proj(cq-1) emitted in the middle of
     chunk cq's attention so the PE never waits on a fresh AllGather.
     Host transposes + concatenates.
"""
import math

import numpy as np
import ml_dtypes

B, T, C, H = 2, 2048, 1024, 16
HD = C // H          # 64 head dim
G = 4                # head-groups (cores per batch)
HPG = H // G         # 4 heads per group
DG = HPG * HD        # 256 dims per group
N_CORES = 8
KC = C // 128        # 8 contraction chunks
NKT = T // 128       # 16 k-tiles
NQC = T // 512       # 4 q-chunks in attention
VS = HD + 2          # V head stride (64 dims + ones col + pad)
RG = [[0, 1, 2, 3], [4, 5, 6, 7]]

_NC_CACHE = {}


def _build():
    import concourse.bacc as bacc
    import concourse.mybir as mybir
    import concourse.tile as tile

    f32 = mybir.dt.float32
    bf16 = mybir.dt.bfloat16
    Exp = mybir.ActivationFunctionType.Exp

    nc = bacc.Bacc("TRN2", num_devices=N_CORES)

    xT_d = nc.dram_tensor("xT", [C, T], bf16, kind="ExternalInput")
    wq_d = nc.dram_tensor("wq", [C, DG], bf16, kind="ExternalInput")
    wk_d = nc.dram_tensor("wk", [C, DG], bf16, kind="ExternalInput")
    wv_d = nc.dram_tensor("wv", [C, DG], bf16, kind="ExternalInput")
    bq_d = nc.dram_tensor("bq", [2, 128, 1], f32, kind="ExternalInput")
    bk_d = nc.dram_tensor("bk", [2, 128, 1], f32, kind="ExternalInput")
    bv_d = nc.dram_tensor("bv", [1, DG], f32, kind="ExternalInput")
    wp_d = nc.dram_tensor("wpTa", [C, DG], bf16, kind="ExternalInput")
    bp_d = nc.dram_tensor("bp", [2, 128, 1], f32, kind="ExternalInput")
    mask_d = nc.dram_tensor("mask", [128, 128], bf16, kind="ExternalInput")
    ones_d = nc.dram_tensor("ones4", [128, HPG, 1], bf16, kind="ExternalInput")
    oT_d = nc.dram_tensor("oT", [DG, T], f32, kind="ExternalOutput")

    with tile.TileContext(nc) as tc:
        with (
            tc.tile_pool(name="persist", bufs=1) as persist,
            tc.tile_pool(name="dram", bufs=1, space="DRAM") as dram,
        ):
            # ---- persistent SBUF ----
            QT = [persist.tile([128, T], bf16, name=f"qt{p}") for p in range(2)]
            KT = [persist.tile([128, T], bf16, name=f"kt{p}") for p in range(2)]
            V1 = [persist.tile([128, HPG * VS], bf16, name=f"v{m}")
                  for m in range(NKT)]
            wpT_sb = [persist.tile([128, DG], bf16, name=f"wp_{k}")
                      for k in range(KC)]
            mask_sb = persist.tile([128, 128], bf16, name="mask_sb")
            bq_sb = [persist.tile([128, 1], f32, name=f"bq{j}") for j in range(2)]
            bk_sb = [persist.tile([128, 1], f32, name=f"bk{j}") for j in range(2)]
            bp_sb = [persist.tile([128, 1], f32, name=f"bp{j}") for j in range(2)]
            bv_row = persist.tile([1, DG], f32, name="bv_row")
            bv_bc = persist.tile([128, DG], f32, name="bv_bc")

            # collective DRAM buffers: merged-pair AGs for chunks 0-2,
            # per-pair AGs for the last chunk (tail overlap)
            yin_m = [dram.tile([256, 512], bf16, name=f"yim{cq}")
                     for cq in range(3)]
            yout_m = [dram.tile([1024, 512], bf16, name=f"yom{cq}")
                      for cq in range(3)]
            yin3 = [dram.tile([128, 512], bf16, name=f"yi3_{p}")
                    for p in range(2)]
            yout3 = [dram.tile([512, 512], bf16, name=f"yo3_{p}")
                     for p in range(2)]
            prime_in = dram.tile([128, 16], bf16, name="prime_in")
            prime_out = dram.tile([512, 16], bf16, name="prime_out")

            # ================= phase 1: QKV =================
            with (
                tc.tile_pool(name="xp", bufs=1) as xp,
                tc.tile_pool(name="wp_s", bufs=1) as wp_s,
                tc.tile_pool(name="qkvps", bufs=1, space="PSUM") as qkvps,
            ):
                # short PE warmup (pstate ramp) while the first loads land
                wu_a = xp.tile([128, 128], bf16, name="wu_a")
                wu_b = xp.tile([128, 512], bf16, name="wu_b")
                nc.vector.memset(wu_a[:], 0.5)
                nc.vector.memset(wu_b[:], 0.5)
                wu_ps = qkvps.tile([128, 512], f32, tag="qkvps", bufs=8,
                                   name="wu_ps")
                for _ in range(8):
                    nc.tensor.matmul(wu_ps[:], wu_a[:], wu_b[:],
                                     start=True, stop=True)

                # input loads: weights on the sync queue; x tiles split
                # across the scalar/vector queues so three HWDGE rings run
                # in parallel (per-ring bandwidth is the startup limit)
                for j in range(2):
                    nc.sync.dma_start(bq_sb[j][:], bq_d[j])
                    nc.sync.dma_start(bk_sb[j][:], bk_d[j])
                    nc.sync.dma_start(bp_sb[j][:], bp_d[j])
                nc.sync.dma_start(bv_row[:], bv_d[:])
                nc.sync.dma_start(mask_sb[:], mask_d[:])

                xT_sb = []
                wq_sb = []
                wk_sb = []
                wv_sb = []
                for k in range(KC):
                    wqt = wp_s.tile([128, DG], bf16, name=f"wq{k}")
                    nc.sync.dma_start(wqt[:], wq_d[128 * k:128 * (k + 1), :])
                    wq_sb.append(wqt)
                    wkt = wp_s.tile([128, DG], bf16, name=f"wk{k}")
                    nc.sync.dma_start(wkt[:], wk_d[128 * k:128 * (k + 1), :])
                    wk_sb.append(wkt)
                    wvt = wp_s.tile([128, DG], bf16, name=f"wv{k}")
                    nc.sync.dma_start(wvt[:], wv_d[128 * k:128 * (k + 1), :])
                    wv_sb.append(wvt)
                    xt = xp.tile([128, T], bf16, name=f"x{k}")
                    eng = nc.scalar if k % 2 == 0 else nc.gpsimd
                    eng.dma_start(xt[:, 0:1024], xT_d[128 * k:128 * (k + 1),
                                                      0:1024])
                    eng.dma_start(xt[:, 1024:2048], xT_d[128 * k:128 * (k + 1),
                                                         1024:2048])
                    xT_sb.append(xt)

                nc.gpsimd.partition_broadcast(bv_bc[:], bv_row[:])
                # priming collective: absorbs the first-op rendezvous skew /
                # CC-stream startup cost while QKV runs, so AG(0) is fast
                nc.gpsimd.collective_compute(
                    "AllGather", mybir.AluOpType.bypass,
                    replica_groups=RG,
                    ins=[prime_in[:].opt()],
                    outs=[prime_out[:].opt()],
                )

                # Q then K: psum [2 jh][4 t4] accumulated over kc
                for sel in range(2):
                    dst = QT if sel == 0 else KT
                    wsb = wq_sb if sel == 0 else wk_sb
                    bcol = bq_sb if sel == 0 else bk_sb
                    ps = [[qkvps.tile([128, 512], f32, tag="qkvps", bufs=8,
                                      name=f"ps{sel}_{jh}_{t4}")
                           for t4 in range(4)] for jh in range(2)]
                    for kc in range(KC):
                        for jh in range(2):
                            for t4 in range(4):
                                nc.tensor.matmul(
                                    ps[jh][t4][:],
                                    wsb[kc][:, 128 * jh:128 * (jh + 1)],
                                    xT_sb[kc][:, 512 * t4:512 * (t4 + 1)],
                                    start=(kc == 0), stop=(kc == KC - 1))
                    # evacuate on DVE and ACT in parallel (bias fused both)
                    for jh in range(2):
                        for t4 in range(4):
                            if t4 % 2 == 0:
                                nc.vector.tensor_scalar_add(
                                    dst[jh][:, 512 * t4:512 * (t4 + 1)],
                                    ps[jh][t4][:], bcol[jh][:])
                            else:
                                nc.scalar.activation(
                                    out=dst[jh][:, 512 * t4:512 * (t4 + 1)],
                                    in_=ps[jh][t4][:],
                                    func=mybir.ActivationFunctionType.Identity,
                                    bias=bcol[jh][:], scale=1.0)

                # V: [t, d] layout, heads at stride VS with ones column
                for mt in range(NKT):
                    psv = qkvps.tile([128, DG], f32, tag="qkvps", bufs=8,
                                     name=f"psv{mt}")
                    for kc in range(KC):
                        nc.tensor.matmul(
                            psv[:],
                            xT_sb[kc][:, 128 * mt:128 * (mt + 1)],
                            wv_sb[kc][:],
                            start=(kc == 0), stop=(kc == KC - 1))
                    vv = V1[mt].rearrange("p (h x) -> p h x", h=HPG)
                    nc.vector.tensor_add(
                        vv[:, :, 0:HD],
                        psv.rearrange("p (h x) -> p h x", h=HPG),
                        bv_bc.rearrange("p (h x) -> p h x", h=HPG))
                    nc.sync.dma_start(vv[:, :, HD:HD + 1], ones_d[:])

            # ============ phase 2: attention / AG / projection ============
            for k in range(KC):
                nc.sync.dma_start(
                    wpT_sb[k][:], wp_d[128 * k:128 * (k + 1), :])

            with (
                tc.tile_pool(name="aps", bufs=1, space="PSUM") as aps,
                tc.tile_pool(name="ppool", bufs=1) as ppool,
                tc.tile_pool(name="npool", bufs=1) as npool,
                tc.tile_pool(name="ynp", bufs=1) as ynp,
                tc.tile_pool(name="yfp", bufs=1) as yfp,
                tc.tile_pool(name="otp", bufs=1) as otp,
            ):
                def emit_attn_pair(cq, p, mid_cb=None, after_norm_cb=None):
                    """S^T -> exp -> mask -> AV -> normalize -> yn tile."""
                    yps = [aps.tile([HD + 1, 512], f32, tag=f"y{X}",
                                    bufs=1, name=f"y_{cq}_{p}_{X}")
                           for X in range(2)]
                    nkt = 4 * (cq + 1)
                    for kt in range(nkt):
                        if kt == 8 and mid_cb is not None:
                            mid_cb()
                        qs = max(0, 128 * kt - 512 * cq)
                        S = aps.tile([128, 1024], f32, tag="s", bufs=2,
                                     name=f"s_{cq}_{p}_{kt}")
                        for X in range(2):
                            nc.tensor.matmul(
                                S[:, 512 * X + qs:512 * (X + 1)],
                                KT[p][64 * X:64 * (X + 1),
                                      128 * kt:128 * (kt + 1)],
                                QT[p][64 * X:64 * (X + 1),
                                      512 * cq + qs:512 * (cq + 1)],
                                start=True, stop=True)
                        Pt = ppool.tile([128, 1024], bf16, tag="p",
                                        bufs=4, name=f"p_{cq}_{p}_{kt}")
                        nc.scalar.activation(
                            out=Pt.rearrange("pp (x q) -> pp x q",
                                             x=2)[:, :, qs:512],
                            in_=S.rearrange("pp (x q) -> pp x q",
                                            x=2)[:, :, qs:512],
                            func=Exp, scale=1.0 / math.sqrt(HD))
                        if kt >= 4 * cq:  # diagonal block: causal mask
                            for X in range(2):
                                nc.vector.tensor_mul(
                                    Pt[:, 512 * X + qs:512 * X + qs + 128],
                                    Pt[:, 512 * X + qs:512 * X + qs + 128],
                                    mask_sb[:])
                        for X in range(2):
                            h = 2 * p + X
                            nc.tensor.matmul(
                                yps[X][:, qs:512],
                                V1[kt][:, VS * h:VS * h + HD + 1],
                                Pt[:, 512 * X + qs:512 * (X + 1)],
                                start=(kt == 0), stop=(kt == nkt - 1))
                    # normalization: denominator row 64 -> 1/x -> broadcast
                    yn = ynp.tile([128, 512], bf16, tag="yn", bufs=4,
                                  name=f"yn_{cq}_{p}")
                    bcs = []
                    for X in range(2):
                        r1r = npool.tile([1, 512], f32, tag="r1r",
                                         bufs=4, name=f"r1r_{cq}_{p}_{X}")
                        nc.vector.tensor_copy(r1r[:], yps[X][HD:HD + 1, :])
                        rro = npool.tile([1, 512], f32, tag="rro",
                                         bufs=4, name=f"rro_{cq}_{p}_{X}")
                        nc.vector.reciprocal_approx_fast(rro[:], r1r[:])
                        bcx = npool.tile([HD, 512], f32, tag="bc",
                                         bufs=4, name=f"bcx_{cq}_{p}_{X}")
                        nc.gpsimd.partition_broadcast(bcx[:], rro[:])
                        bcs.append(bcx)
                    if after_norm_cb is not None:
                        # deferred AG trigger rides the gpsimd queue AFTER
                        # this pair's broadcasts so its input-wait never
                        # delays them (they free the PSUM accumulators)
                        after_norm_cb()
                    for X in range(2):
                        nc.vector.tensor_mul(
                            yn[64 * X:64 * (X + 1), :],
                            yps[X][0:HD, :], bcs[X][:])
                    return yn

                def emit_yfs(tq, kds, eng):
                    out = {}
                    for kd in kds:
                        yf = yfp.tile([128, 512], bf16, tag="yf", bufs=8,
                                      name=f"yf_{tq}_{kd}")
                        g2, p2 = divmod(kd, 2)
                        if tq == 3:
                            src = yout3[p2][128 * g2:128 * (g2 + 1), :]
                        else:
                            src = yout_m[tq][256 * g2 + 128 * p2:
                                             256 * g2 + 128 * p2 + 128, :]
                        eng.dma_start(yf[:], src)
                        out[kd] = yf
                    return out

                def emit_proj(tq, yfs=None):
                    """o^T[:, 512tq:512(tq+1)] from the AllGathered y^T."""
                    if tq == 3:
                        order = [0, 2, 4, 6, 1, 3, 5, 7]
                    else:
                        order = list(range(KC))
                    if yfs is None:
                        yfs = emit_yfs(tq, order, nc.sync)
                    po = [aps.tile([128, 512], f32, tag="po", bufs=2,
                                   name=f"po_{tq}_{eh}") for eh in range(2)]
                    for i, kd in enumerate(order):
                        for eh in range(2):
                            nc.tensor.matmul(
                                po[eh][:],
                                wpT_sb[kd][:, 128 * eh:128 * (eh + 1)],
                                yfs[kd][:],
                                start=(i == 0), stop=(i == KC - 1))
                    for eh in range(2):
                        ot = otp.tile([128, 512], f32, tag="ot", bufs=2,
                                      name=f"ot_{tq}_{eh}")
                        nc.vector.tensor_scalar_add(ot[:], po[eh][:],
                                                    bp_sb[eh][:])
                        nc.sync.dma_start(
                            oT_d[128 * eh:128 * (eh + 1),
                                 512 * tq:512 * (tq + 1)], ot[:])

                def make_ag(i, o):
                    def ag():
                        nc.gpsimd.collective_compute(
                            "AllGather", mybir.AluOpType.bypass,
                            replica_groups=RG, ins=[i[:].opt()],
                            outs=[o[:].opt()])
                    return ag

                yfs3 = {}
                pending_ag = [None]

                def mid3():
                    # mid-way through the last pair: proj(2), plus the
                    # already-gathered half of proj(3)'s inputs
                    emit_proj(2)
                    yfs3.update(emit_yfs(3, [0, 2, 4, 6], nc.scalar))

                for cq in range(NQC):
                    last = cq == NQC - 1
                    for p in range(2):
                        mid = mid3 if (last and p == 1) else None
                        anc = pending_ag[0] if p == 0 else None
                        pending_ag[0] = None if p == 0 else pending_ag[0]
                        yn = emit_attn_pair(cq, p, mid_cb=mid,
                                            after_norm_cb=anc)
                        if last:
                            nc.sync.dma_start(yin3[p][:], yn[:])
                            make_ag(yin3[p], yout3[p])()
                        else:
                            nc.sync.dma_start(
                                yin_m[cq][128 * p:128 * (p + 1), :], yn[:])
                        if p == 0 and cq >= 2:
                            emit_proj(cq - 2)
                    if not last:
                        pending_ag[0] = make_ag(yin_m[cq], yout_m[cq])
                yfs3.update(emit_yfs(3, [1, 3, 5, 7], nc.scalar))
                emit_proj(3, yfs3)

    nc.finalize()
    return nc


def _get_nc():
    if "nc" not in _NC_CACHE:
        _NC_CACHE["nc"] = _build()
    return _NC_CACHE["nc"]


def kernel(x, w_attn, b_attn, w_proj, b_proj):
    from concourse.bass_utils import run_bass_kernel_spmd

    x = np.asarray(x, dtype=np.float32)
    w_attn = np.asarray(w_attn, dtype=np.float32)
    b_attn = np.asarray(b_attn, dtype=np.float32)
    w_proj = np.asarray(w_proj, dtype=np.float32)
    b_proj = np.asarray(b_proj, dtype=np.float32)

    mask = np.triu(np.ones((128, 128), dtype=np.float32)).copy()

    in_maps = []
    for c in range(N_CORES):
        b, g = divmod(c, G)
        lo = DG * g
        wpT = np.ascontiguousarray(w_proj[lo:lo + DG, :].T)
        in_maps.append({
            "xT": np.ascontiguousarray(x[b].T).astype(ml_dtypes.bfloat16),
            "wq": np.ascontiguousarray(w_attn[lo:lo + DG, :].T).astype(ml_dtypes.bfloat16),
            "wk": np.ascontiguousarray(w_attn[C + lo:C + lo + DG, :].T).astype(ml_dtypes.bfloat16),
            "wv": np.ascontiguousarray(w_attn[2 * C + lo:2 * C + lo + DG, :].T).astype(ml_dtypes.bfloat16),
            "bq": np.ascontiguousarray(b_attn[lo:lo + DG].reshape(2, 128, 1)),
            "bk": np.ascontiguousarray(
                b_attn[C + lo:C + lo + DG].reshape(2, 128, 1)),
            "bv": np.ascontiguousarray(
                b_attn[2 * C + lo:2 * C + lo + DG].reshape(1, DG)),
            "wpTa": wpT.astype(ml_dtypes.bfloat16),
            "bp": np.ascontiguousarray(b_proj[lo:lo + DG].reshape(2, 128, 1)),
            "mask": mask.astype(ml_dtypes.bfloat16),
            "ones4": np.ones((128, HPG, 1), dtype=ml_dtypes.bfloat16),
        })

    global _last_in_maps
    _last_in_maps = in_maps

    nc = _get_nc()
    res = run_bass_kernel_spmd(nc, in_maps, list(range(N_CORES)))

    out = np.empty((B, T, C), dtype=np.float32)
    for c in range(N_CORES):
        b, g = divmod(c, G)
        out[b, :, DG * g:DG * (g + 1)] = res.results[c]["oT"].T
    return out
